# revision 2
# baseline (speedup 1.0000x reference)
"""Differential multi-head attention (DiffAttn) Trainium2 Bass kernel.

Math (per batch b, head h):
  lam      = exp(<lq1,lk1>) - exp(<lq2,lk2>) + LAMBDA_INIT          (scalar)
  logits1  = Q  K^T  / sqrt(64);  logits2 = Q2 K2^T / sqrt(64)      [S,S]
  attn     = softmax(logits1) - lam * softmax(logits2)
  out[b,:,h*64:(h+1)*64] = attn @ V                                  [S,64]

Device strategy: 64 (b,h) pairs sharded 8-per-core across 8 NeuronCores
(pure data parallel, no collectives). Per pair, everything is computed in
the *transposed* logits layout E[k, q] so that:
  - the two streams (Q,K) and (Q2,K2) pack into one 128-row contraction
    (row-group tile_position packing) for the QK matmuls,
  - softmax denominators come free from a ones-column appended to V in
    the PV matmul (PE does the partition-dim reduction),
  - no on-chip transposes are needed anywhere (host pre-transposes Q/K
    and post-transposes the [64, S] per-pair output; layout-only work).
Normalization 1/s is broadcast across partitions with a tiny ones-matmul.
exp() runs on ScalarE reading PSUM [128,1024] tiles (ACT is the bottleneck
engine: 2*S*S elements per pair).
"""

import math
import os

import numpy as np

import concourse.bass as bass  # noqa: F401  (bass types via bacc)
import concourse.mybir as mybir
import concourse.tile as tile
from concourse import bacc
from concourse.bass_utils import run_bass_kernel_spmd

B, H, S, DK, DV = 4, 16, 1024, 64, 64
N_CORES = 8
PAIRS = (B * H) // N_CORES  # 8 (b,h) pairs per core
KT = S // 128  # 8 k-tiles of 128
NQ = S // 512  # 2 q-chunks of 512
VA = DV + 1  # V columns + ones column
LAMBDA_INIT = 0.8 - 0.6 * math.exp(-0.3 * 10)

dt = mybir.dt


def build_nc(pairs: int = PAIRS, reps: int = 1):
    """Build the SPMD Bass program (same NEFF on all cores)."""
    nc = bacc.Bacc(
        "TRN2", target_bir_lowering=False, debug=False, num_devices=N_CORES
    )

    # DRAM I/O (per-core shapes). float32r is bit-identical to float32; it
    # selects the full-rate PE path (plain fp32 matmul is 4 cycles/row).
    qt_d = nc.dram_tensor("qt", [pairs, 128, S], dt.float32r, kind="ExternalInput")
    kt_d = nc.dram_tensor("kt", [pairs, 128, S], dt.float32r, kind="ExternalInput")
    v1_d = nc.dram_tensor("v1", [pairs, 128, KT * VA], dt.float16, kind="ExternalInput")
    v2_d = nc.dram_tensor("v2", [pairs, 128, KT * VA], dt.float16, kind="ExternalInput")
    o_d = nc.dram_tensor("o", [pairs, DV, S], dt.float32, kind="ExternalOutput")
    qt_ap, kt_ap, v1_ap, v2_ap, o_ap = (
        qt_d.ap(), kt_d.ap(), v1_d.ap(), v2_d.ap(), o_d.ap()
    )

    with tile.TileContext(nc) as tc:
        with (
            tc.tile_pool(name="const", bufs=1) as constp,
            tc.tile_pool(name="qk", bufs=2) as qkp,
            tc.tile_pool(name="vp", bufs=2) as vp,
            tc.tile_pool(name="ep", bufs=3) as ep,
            tc.tile_pool(name="psE", bufs=2, space="PSUM") as psE,
            tc.tile_pool(name="psU", bufs=1, space="PSUM") as psU,
            tc.tile_pool(name="psR", bufs=1, space="PSUM") as psR,
            tc.tile_pool(name="cmb", bufs=2) as cmb,
            tc.tile_pool(name="outp", bufs=2) as outp,
        ):
            ones = constp.tile([1, DV], dt.float32, name="ones")
            nc.vector.memset(ones, 1.0)

            for _ in range(reps):
                for p in range(pairs):
                    qt = qkp.tile([128, S], dt.float32r, tag="qt", name="qt_sb")
                    kt = qkp.tile([128, S], dt.float32r, tag="kt", name="kt_sb")
                    v1 = vp.tile([128, KT * VA], dt.float16, tag="v1", name="v1_sb")
                    v2 = vp.tile([128, KT * VA], dt.float16, tag="v2", name="v2_sb")
                    nc.sync.dma_start(qt, qt_ap[p])
                    nc.sync.dma_start(kt, kt_ap[p])
                    nc.sync.dma_start(v1, v1_ap[p])
                    nc.sync.dma_start(v2, v2_ap[p])

                    outT = outp.tile([DV, S], dt.float32, tag="outT", name="outT")

                    for n in range(NQ):
                        nsl = slice(n * 512, (n + 1) * 512)
                        u1 = psU.tile([VA, 512], dt.float32, tag="u1", name="u1")
                        u2 = psU.tile([VA, 512], dt.float32, tag="u2", name="u2")
                        for k in range(KT):
                            ksl = slice(k * 128, (k + 1) * 128)
                            e_ps = psE.tile([128, 1024], dt.float32, tag="e", name="e_ps")
                            # logits^T for stream 1 (rows 0:64 of qt/kt) and
                            # stream 2 (rows 64:128) — concurrent row-groups.
                            nc.tensor.matmul(
                                e_ps[:, 0:512], kt[0:64, ksl], qt[0:64, nsl],
                                start=True, stop=True,
                            )
                            nc.tensor.matmul(
                                e_ps[:, 512:1024], kt[64:128, ksl], qt[64:128, nsl],
                                start=True, stop=True, tile_position=(64, 0),
                            )
                            e_sb = ep.tile([128, 1024], dt.float16, tag="e_sb", name="e_sb")
                            nc.scalar.activation(
                                e_sb, e_ps, mybir.ActivationFunctionType.Exp
                            )
                            # PV accumulate: U = [V|1]^T @ E  (row 64 = sums)
                            nc.tensor.matmul(
                                u1, v1[:, k * VA:(k + 1) * VA], e_sb[:, 0:512],
                                start=(k == 0), stop=(k == KT - 1),
                            )
                            nc.tensor.matmul(
                                u2, v2[:, k * VA:(k + 1) * VA], e_sb[:, 512:1024],
                                start=(k == 0), stop=(k == KT - 1),
                            )
                        # r = 1/s ;  R = broadcast(r) over 64 partitions (PE)
                        r1 = cmb.tile([1, 512], dt.float32, tag="r1", name="r1")
                        r2 = cmb.tile([1, 512], dt.float32, tag="r2", name="r2")
                        nc.vector.reciprocal(r1, u1[DV:VA, :])
                        nc.vector.reciprocal(r2, u2[DV:VA, :])
                        R1 = psR.tile([DV, 512], dt.float32, tag="R1", name="R1")
                        R2 = psR.tile([DV, 512], dt.float32, tag="R2", name="R2")
                        nc.tensor.matmul(R1, ones, r1, start=True, stop=True)
                        nc.tensor.matmul(R2, ones, r2, start=True, stop=True)
                        R1s = cmb.tile([DV, 512], dt.float32, tag="R1s", name="R1s")
                        R2s = cmb.tile([DV, 512], dt.float32, tag="R2s", name="R2s")
                        nc.vector.tensor_copy(R1s, R1)
                        nc.vector.tensor_copy(R2s, R2)
                        m1 = cmb.tile([DV, 512], dt.float32, tag="m1", name="m1")
                        m2 = cmb.tile([DV, 512], dt.float32, tag="m2", name="m2")
                        nc.vector.tensor_mul(m1, u1[0:DV, :], R1s)
                        nc.vector.tensor_mul(m2, u2[0:DV, :], R2s)
                        nc.vector.tensor_sub(outT[:, nsl], m1, m2)

                    nc.sync.dma_start(o_ap[p], outT)

    nc.compile()
    return nc


def prepare_inputs(key, query, value, differential_key, differential_query,
                   lambda_q1, lambda_k1, lambda_q2, lambda_k2):
    """Host-side shard + layout packing (layout-only work + per-head scalar
    lambda). Returns in_maps for the 8 cores."""
    scale = 1.0 / math.sqrt(DK)
    lam = (
        np.exp(np.dot(np.asarray(lambda_q1, np.float64),
                      np.asarray(lambda_k1, np.float64)))
        - np.exp(np.dot(np.asarray(lambda_q2, np.float64),
                        np.asarray(lambda_k2, np.float64)))
        + LAMBDA_INIT
    )  # scalar

    q = np.asarray(query, np.float32).reshape(B * H, S, DK)
    q2 = np.asarray(differential_query, np.float32).reshape(B * H, S, DK)
    k = np.asarray(key, np.float32).reshape(B * H, S, DK)
    k2 = np.asarray(differential_key, np.float32).reshape(B * H, S, DK)
    v = np.asarray(value, np.float32).reshape(B * H, S, DV)

    # qt[g] = [ (Q/8)^T ; (Q2/8)^T ]  -> [128, S]
    qt = np.concatenate(
        [np.transpose(q, (0, 2, 1)) * scale, np.transpose(q2, (0, 2, 1)) * scale],
        axis=1,
    ).astype(np.float32)  # [64, 128, S]
    kt = np.concatenate(
        [np.transpose(k, (0, 2, 1)), np.transpose(k2, (0, 2, 1))], axis=1
    ).astype(np.float32)

    ones_col = np.ones((B * H, KT, 128, 1), np.float32)

    def pack_v(vscaled):
        # [g, S, DV] -> [g, KT, 128, DV] -> append ones -> [g, 128, KT*VA]
        vt = vscaled.reshape(B * H, KT, 128, DV)
        vt = np.concatenate([vt, ones_col], axis=-1)  # [g, KT, 128, VA]
        vt = np.transpose(vt, (0, 2, 1, 3)).reshape(B * H, 128, KT * VA)
        return vt.astype(np.float16)

    v1 = pack_v(v)
    lam_g = np.repeat(lam.reshape(1), B * H).reshape(B * H, 1, 1).astype(np.float64)
    v2 = pack_v((v.astype(np.float64) * lam_g).astype(np.float32))

    in_maps = []
    for c in range(N_CORES):
        sl = slice(c * PAIRS, (c + 1) * PAIRS)
        in_maps.append({
            "qt": np.ascontiguousarray(qt[sl]),
            "kt": np.ascontiguousarray(kt[sl]),
            "v1": np.ascontiguousarray(v1[sl]),
            "v2": np.ascontiguousarray(v2[sl]),
        })
    return in_maps


def assemble_output(results):
    """results: list of 8 dicts with 'o' [PAIRS, DV, S] -> [B, S, H*DV].

    The reference reshapes [B,H,S,Dv] -> (B, S, H*Dv) with a *plain* reshape
    (torch .view semantics), so replicate that exactly."""
    bhsv = np.empty((B, H, S, DV), np.float32)
    for c in range(N_CORES):
        o = results[c]["o"]  # [PAIRS, 64, S]
        for p in range(PAIRS):
            g = c * PAIRS + p
            bhsv[g // H, g % H] = o[p].T
    return bhsv.reshape(B, S, H * DV)


_NC_CACHE = {}


def _get_nc():
    if "nc" not in _NC_CACHE:
        _NC_CACHE["nc"] = build_nc(PAIRS, reps=int(os.environ.get("KERNEL_REPS", "1")))
    return _NC_CACHE["nc"]


def kernel(**inputs) -> np.ndarray:
    nc = _get_nc()
    in_maps = prepare_inputs(**inputs)
    res = run_bass_kernel_spmd(nc, in_maps, core_ids=list(range(N_CORES)))
    return assemble_output(res.results)
